# revision 22
# baseline (speedup 1.0000x reference)
"""Cached multi-head attention (decode-append, S=4) on 8 Trainium2 NeuronCores.

Sharding: tensor-parallel over the 32 heads -> 4 heads per core.
  - Wq/Wk/Wv split on the output-feature (head) axis, Wo on the input axis.
  - Each core holds its heads' slice of the KV cache.
  - Each core produces a partial output; the all-reduce is a host-side sum.

Precision/layout strategy (v2):
  - K and V caches stored as fp8 e3m4 (4-bit mantissa), pre-scaled on the
    host into e3m4's range (alpha_k, alpha_v).  The PE eats them directly as
    stationary weights (mixed fp8 x fp16 matmuls) -- no on-chip dequant.
    Descales fold into the exp scale constant (1/alpha_k^2, since Wq is also
    host-scaled by alpha_k) and host-folded Wo rows (1/alpha_v).  Wk/Wv are
    stored as e3m4 * alpha_{k,v} so the on-chip k_new/v_new match the cache
    scaling exactly; their quantization noise only touches the 4 new
    positions (~0.1% of the attention mass).
  - Scores: lhsT = K^T-tile [hd=128, kv=128] e3m4 stationary (~48ns/LD),
    rhs = qT [hd, 4 tok] fp16 -> scores^T [kv, tok] in PSUM, s-major cols.
  - PV flipped: lhsT = V-tile [kv=128, feat=128] e3m4, rhs = probs [kv, 4]
    -> out4 [feat, tok] accumulated over kv tiles in PSUM.  No transposes,
    no 129-column moving streams.
  - Denominators: per (b,h) one matmul ones[128,1]^T @ probs -> [1,128],
    DVE strided reduce -> [1,4]; new-token part via ones32 over pn_all.
    Normalize at the end: reciprocal [1,128], PE row-broadcast, one DVE
    tensor_mul over all 128 (head, token) columns.
  - q/k projections flipped: W^T tiles [in=128, out=128] stationary, xT
    moving -> qT/kT feature-major directly.  v-proj x-stationary -> v_tok
    token-major for the new-token PV; pn_all [32,32] with a block-diagonal
    causal mask kills cross-batch terms in one matmul per head.
  - o_proj flipped: Wo^T tiles stationary, attnT moving -> out^T [H, tok];
    host transposes and sums partials across cores.
"""

import numpy as np
import ml_dtypes

import concourse.bacc as bacc
import concourse.mybir as mybir
import concourse.tile as tile
from concourse.bass_utils import run_bass_kernel_spmd

N_CORES = 8
B, S, H = 8, 4, 4096
NH = 32                 # total heads
HPC = NH // N_CORES     # heads per core = 4
HD = H // NH            # head dim = 128
POS = 4096              # cache positions attended
NT = POS // 128         # kv tiles per (b, h) = 32
NTOK = B * S            # 32 query tokens, token index = 4*b + s
KPC = HPC * HD          # per-core feature slice = 512
SCALE = HD ** -0.5
NEG = -1e9

F16 = mybir.dt.float16
F32 = mybir.dt.float32
E3 = mybir.dt.float8e3
E3NP = ml_dtypes.float8_e3m4

NT16 = 8             # V tiles kept in fp16 (rest e3m4)
E3_TARGET = 14.0        # e3m4 max normal is 15.5; leave rounding headroom
V_FP16 = True           # V cache in fp16 (safe accuracy) vs e3m4 (fast DMA)


def build_nc(alpha_k, gk, gv):
    """alpha_k bakes into the exp scale, gk/gv into the k/v copy scales."""
    nc = bacc.Bacc("TRN2", target_bir_lowering=False)
    VDT = F16 if V_FP16 else E3

    xT = nc.dram_tensor("xT", [128, NT * NTOK], F16, kind="ExternalInput")
    wq = nc.dram_tensor("wq", [128, NT * KPC], F16, kind="ExternalInput")
    wk = nc.dram_tensor("wk", [128, NT * KPC], E3, kind="ExternalInput")
    wv = nc.dram_tensor("wv", [128, NT * KPC], E3, kind="ExternalInput")
    wo = nc.dram_tensor("wo", [128, HPC * H], F16, kind="ExternalInput")
    kt = nc.dram_tensor("kt", [B * HPC, 128, POS], E3, kind="ExternalInput")
    v = nc.dram_tensor("v", [B * HPC, 128, NT16 * 128], F16, kind="ExternalInput")
    v8 = nc.dram_tensor("v8", [B * HPC, 128, (NT - NT16) * 128], E3, kind="ExternalInput")
    mask = nc.dram_tensor("mask", [NTOK, NTOK], F32, kind="ExternalInput")
    out = nc.dram_tensor("out", [128, NT * NTOK], F32, kind="ExternalOutput")

    with tile.TileContext(nc) as tc:
        _body(tc, xT.ap(), wq.ap(), wk.ap(), wv.ap(), wo.ap(), kt.ap(), v.ap(),
              v8.ap(), mask.ap(), out.ap(), alpha_k, gk, gv)
    nc.compile()
    return nc


def _body(tc, xT, wq, wk, wv, wo, kt, v, v8, mask, out, alpha_k, gk, gv):
    nc = tc.nc
    from contextlib import ExitStack
    Exp = mybir.ActivationFunctionType.Exp
    Copy = mybir.ActivationFunctionType.Copy
    VDT = F16 if V_FP16 else E3
    ESCALE = SCALE / (alpha_k * alpha_k)  # scores carry alpha_k^2

    ctx = ExitStack()
    with ctx:
        consts = ctx.enter_context(tc.tile_pool(name="consts", bufs=1))
        persist = ctx.enter_context(tc.tile_pool(name="persist", bufs=1))
        wpool = ctx.enter_context(tc.tile_pool(name="wpool", bufs=1))
        kvpool = ctx.enter_context(tc.tile_pool(name="kvpool", bufs=6))
        smpool = ctx.enter_context(tc.tile_pool(name="smpool", bufs=3))
        ps = ctx.enter_context(tc.tile_pool(name="ps", bufs=2, space="PSUM"))

        # ---- constants / persistent state ----
        ones128 = consts.tile([128, 1], F16)
        nc.vector.memset(ones128, 1.0)
        ones32 = consts.tile([32, 1], F16)
        nc.vector.memset(ones32, 1.0)
        ones_row = consts.tile([1, HPC * NTOK], F32)
        nc.vector.memset(ones_row, 1.0)
        mask_sb = consts.tile([NTOK, NTOK], F32)
        nc.sync.dma_start(out=mask_sb, in_=mask)

        xT_sb = persist.tile([128, NT * NTOK], F16)
        nc.sync.dma_start(out=xT_sb, in_=xT)
        # weights ride the SWDGE ring so KV streaming owns the sync ring;
        # chunked so the projection chains (which consume t-major tiles)
        # start as soon as the first chunk lands
        wq_sb = wpool.tile([128, NT * KPC], F16)
        wk_sb = wpool.tile([128, NT * KPC], E3)
        wv_sb = wpool.tile([128, NT * KPC], E3)
        WCH = NT * KPC // 4
        for c4 in range(4):
            wsl = slice(WCH * c4, WCH * (c4 + 1))
            nc.gpsimd.dma_start(out=wq_sb[:, wsl], in_=wq[:, wsl])
            nc.gpsimd.dma_start(out=wk_sb[:, wsl], in_=wk[:, wsl])
            nc.gpsimd.dma_start(out=wv_sb[:, wsl], in_=wv[:, wsl])

        # KV prefetch ring (h-major: i = h*B + b so o_proj can run per-head)
        kvch = {}

        def fetch_kv(i):
            kc = kvpool.tile([128, POS], E3, tag="kt", name=f"kt{i}")
            nc.sync.dma_start(out=kc, in_=kt[i])
            vc = kvpool.tile([128, NT16 * 128], F16, tag="v", name=f"v{i}")
            nc.sync.dma_start(out=vc, in_=v[i])
            vc8 = kvpool.tile([128, (NT - NT16) * 128], E3, tag="v8", name=f"v8{i}")
            nc.sync.dma_start(out=vc8, in_=v8[i])
            kvch[i] = (kc, vc, vc8)

        fetch_kv(0)
        fetch_kv(1)
        fetch_kv(2)
        fetch_kv(3)
        fetch_kv(4)
        wo_sb = wpool.tile([128, HPC * H], F16)
        nc.gpsimd.dma_start(out=wo_sb, in_=wo)

        # ---- phase 1: projections ----
        # q/k flipped: W^T tile [in 128, feat 128] stationary, xT moving
        qT_sb = persist.tile([128, HPC * NTOK], F16)
        kT_sb = persist.tile([128, HPC * NTOK], F16)
        v_tok = persist.tile([NTOK, KPC], F16)

        for w_sb, dst, dsc in ((wq_sb, qT_sb, None), (wk_sb, kT_sb, 1.0 / gk)):
            for h in range(HPC):
                pp = ps.tile([128, NTOK], F32, tag="sc", name=f"pj{h}", bufs=3)
                for t in range(NT):
                    nc.tensor.matmul(
                        pp,
                        lhsT=w_sb[:, KPC * t + HD * h: KPC * t + HD * (h + 1)],
                        rhs=xT_sb[:, NTOK * t: NTOK * (t + 1)],
                        start=(t == 0), stop=(t == NT - 1))
                if dsc is None:
                    nc.scalar.copy(out=dst[:, NTOK * h: NTOK * (h + 1)], in_=pp)
                else:
                    nc.scalar.activation(out=dst[:, NTOK * h: NTOK * (h + 1)],
                                         in_=pp, func=Copy, scale=float(dsc))

        # v-proj x-stationary: xT tile stationary, Wv moving -> [tok, feat]
        vp = ps.tile([NTOK, KPC], F32, tag="sc", bufs=3)
        for t in range(NT):
            nc.tensor.matmul(
                vp, lhsT=xT_sb[:, NTOK * t: NTOK * (t + 1)],
                rhs=wv_sb[:, KPC * t: KPC * (t + 1)],
                start=(t == 0), stop=(t == NT - 1))
        nc.scalar.activation(out=v_tok, in_=vp, func=Copy, scale=float(1.0 / gv))

        # pn_all per head: [32 kv-new, 32 tok] with block-diag causal mask
        pn_all = [persist.tile([NTOK, NTOK], F16, name=f"pn{h}") for h in range(HPC)]
        dn_sb = persist.tile([1, HPC * NTOK], F32)
        for h in range(HPC):
            sn = ps.tile([NTOK, NTOK], F32, tag="dd", bufs=2)
            nc.tensor.matmul(sn, lhsT=kT_sb[:, NTOK * h: NTOK * (h + 1)],
                             rhs=qT_sb[:, NTOK * h: NTOK * (h + 1)],
                             start=True, stop=True)
            snm = smpool.tile([NTOK, NTOK], F32, tag="snm", bufs=2)
            nc.vector.tensor_add(out=snm, in0=sn, in1=mask_sb)
            nc.scalar.activation(out=pn_all[h], in_=snm, func=Exp, scale=ESCALE)
            dnp = ps.tile([1, NTOK], F32, tag="dd", bufs=2)
            nc.tensor.matmul(dnp, lhsT=ones32, rhs=pn_all[h], start=True, stop=True)
            nc.scalar.copy(out=dn_sb[:, NTOK * h: NTOK * (h + 1)], in_=dnp)

        # ---- phase 2: attention over the cache, per (b, h) ----
        attnT = persist.tile([128, HPC * NTOK], F16)
        o_all = persist.tile([128, NT * NTOK], F32)
        out4_all = persist.tile([128, HPC * NTOK], F32)
        den_all = persist.tile([1, HPC * NTOK], F32)

        for i in range(B * HPC):
            h, b = divmod(i, B)
            if i + 5 < B * HPC:
                fetch_kv(i + 5)
            kc, vc, vc8 = kvch.pop(i)
            col = NTOK * h + S * b  # (head, token) column in qT/out4/den

            # scores^T [kv 128, (s,t) 128] s-major: col = s*32 + t
            sc_ps = ps.tile([128, S * NT], F32, tag="sc", bufs=3)
            sc3 = sc_ps.rearrange("p (s t) -> p s t", t=NT)
            for t in range(NT):
                nc.tensor.matmul(
                    sc3[:, :, t: t + 1],
                    lhsT=kc[:, 128 * t: 128 * (t + 1)],
                    rhs=qT_sb[:, col: col + S],
                    start=True, stop=True)
            probs = smpool.tile([128, S * NT], F16, tag="probs", bufs=3)
            nc.scalar.activation(out=probs, in_=sc_ps, func=Exp, scale=ESCALE)
            pr3 = probs.rearrange("p (s t) -> p s t", t=NT)

            # denominator: ones^T @ probs -> [1, 128], reduce t, add new part
            dd = ps.tile([1, S * NT], F32, tag="dd", bufs=2)
            nc.tensor.matmul(dd, lhsT=ones128, rhs=probs, start=True, stop=True)
            dtmp = smpool.tile([1, S], F32, tag="dtmp", bufs=2)
            nc.vector.reduce_sum(
                out=dtmp, in_=dd.rearrange("p (s t) -> p s t", t=NT),
                axis=mybir.AxisListType.X)
            nc.vector.tensor_add(out=den_all[:, col: col + S], in0=dtmp,
                                 in1=dn_sb[:, col: col + S])

            # PV flipped: V-tile stationary [kv, feat], probs moving [kv, 4]
            o4 = ps.tile([128, S], F32, tag="o4", bufs=3)
            for t in range(NT):
                vt = (vc[:, 128 * t: 128 * (t + 1)] if t < NT16
                      else vc8[:, 128 * (t - NT16): 128 * (t - NT16 + 1)])
                nc.tensor.matmul(
                    o4, lhsT=vt, rhs=pr3[:, :, t: t + 1],
                    start=(t == 0), stop=False)
            nc.tensor.matmul(
                o4, lhsT=v_tok[:, HD * h: HD * (h + 1)],
                rhs=pn_all[h][:, S * b: S * (b + 1)],
                start=False, stop=True)
            nc.scalar.copy(out=out4_all[:, col: col + S], in_=o4)

            # head wave: batched normalize; o_proj chunks are deferred and
            # drip-fed into later iterations so they never monopolize the
            # in-order PE queue
            if b == B - 1:
                hsl = slice(NTOK * h, NTOK * (h + 1))
                rec32 = smpool.tile([1, NTOK], F32, tag="rec", bufs=2)
                nc.vector.reciprocal(out=rec32, in_=den_all[:, hsl])
                rb = ps.tile([128, NTOK], F32, tag="dd", bufs=2)
                nc.tensor.matmul(rb, lhsT=ones_row, rhs=rec32, start=True, stop=True)
                rb_sb = smpool.tile([128, NTOK], F32, tag="rb", bufs=2)
                nc.scalar.copy(out=rb_sb, in_=rb)
                nc.vector.tensor_mul(out=attnT[:, hsl], in0=out4_all[:, hsl],
                                     in1=rb_sb)
                for oc in range(NT):
                    op = ps.tile([128, NTOK], F32, tag="sc", bufs=3)
                    nc.tensor.matmul(
                        op,
                        lhsT=wo_sb[:, H * h + 128 * oc: H * h + 128 * (oc + 1)],
                        rhs=attnT[:, NTOK * h: NTOK * (h + 1)],
                        start=True, stop=True)
                    osl = o_all[:, NTOK * oc: NTOK * (oc + 1)]
                    if h == 0:
                        nc.scalar.copy(out=osl, in_=op)
                    else:
                        nc.vector.tensor_add(out=osl, in0=osl, in1=op)
        nc.sync.dma_start(out=out, in_=o_all)


# ---------------------------------------------------------------------------
# host side
# ---------------------------------------------------------------------------

def _scales(key_cache, value_cache):
    ak = E3_TARGET / max(float(np.abs(key_cache[:, :, :POS]).max()), 1e-6)
    av = E3_TARGET / max(float(np.abs(value_cache[:, :, :POS]).max()), 1e-6)
    return ak, av


def build_core_inputs(hidden_states, Wq, Wk, Wv, Wo, key_cache, value_cache):
    """Shard + lay out the full inputs into the 8 per-core DRAM images."""
    ak, av = _scales(key_cache, value_cache)

    tokens = np.ascontiguousarray(hidden_states.reshape(NTOK, H))
    xT = tokens.T.astype(np.float16)                       # [4096, 32]
    xT_sb = np.ascontiguousarray(
        xT.reshape(NT, 128, NTOK).transpose(1, 0, 2)).reshape(128, NT * NTOK)

    WqT = Wq.T.astype(np.float32) * ak                     # [in, out] * ak
    # k_new/v_new must match the cache scaling (ak / av); gk, gv lift the
    # e3m4-stored weights out of the denormal range and are divided back out
    # by the on-chip psum->sbuf copy scales.
    WkTs = Wk.T.astype(np.float32) * ak
    gk = E3_TARGET / max(float(np.abs(WkTs).max()), 1e-30)
    WkT = WkTs * gk
    WvTs = Wv.T.astype(np.float32) * av
    gv = E3_TARGET / max(float(np.abs(WvTs).max()), 1e-30)
    WvT = WvTs * gv
    WoT = Wo.T.astype(np.float32) / av                     # undo av after PV

    def wlayout_flip(WT, dt):
        # [4096 in, 512 out] -> [128, (t_in 32, 512)] tiles [in128, out...]
        a = np.ascontiguousarray(WT).reshape(NT, 128, KPC)
        return np.ascontiguousarray(a.transpose(1, 0, 2)).reshape(128, NT * KPC).astype(dt)

    K8 = (key_cache[:, :, :POS].astype(np.float32) * ak).astype(E3NP)
    Vs = value_cache[:, :, :POS].astype(np.float32) * av

    # block-diagonal causal mask for pn_all [kv-new i, tok j]
    mask = np.full((NTOK, NTOK), np.float32(NEG), np.float32)
    for b in range(B):
        for i_ in range(S):
            for j in range(i_, S):
                mask[S * b + i_, S * b + j] = 0.0

    in_maps = []
    for c in range(N_CORES):
        cs = slice(KPC * c, KPC * (c + 1))
        hs = slice(HPC * c, HPC * (c + 1))

        wq_c = wlayout_flip(WqT[:, cs], np.float16)
        wk_c = wlayout_flip(WkT[:, cs], E3NP)
        wv_c = wlayout_flip(WvT[:, cs], E3NP)

        # wo: [512 in, 4096 out] rows slice -> [128, (h 4, oc*128+o)]
        wo_c = np.ascontiguousarray(WoT[cs, :]).reshape(HPC, 128, H)
        wo_c = np.ascontiguousarray(wo_c.transpose(1, 0, 2)).reshape(128, HPC * H)
        wo_c = wo_c.astype(np.float16)

        # kt: K^T blocks in fetch order i = h*B + b: [hd 128, kv 4096]
        kt_c = np.ascontiguousarray(
            K8[:, hs].transpose(1, 0, 3, 2)).reshape(B * HPC, 128, POS)
        # v: [kv-in-tile 128, (t, f)] blocks in fetch order; tiles 0..15
        # stay fp16, tiles 16..31 go e3m4
        v_all = np.ascontiguousarray(
            Vs[:, hs].reshape(B, HPC, NT, 128, HD).transpose(1, 0, 3, 2, 4)
        ).reshape(B * HPC, 128, NT, HD)
        v_c = np.ascontiguousarray(
            v_all[:, :, :NT16]).reshape(B * HPC, 128, NT16 * 128).astype(np.float16)
        v8_c = np.ascontiguousarray(
            v_all[:, :, NT16:]).reshape(B * HPC, 128, (NT - NT16) * 128).astype(E3NP)

        in_maps.append({
            "xT": xT_sb, "wq": wq_c, "wk": wk_c, "wv": wv_c, "wo": wo_c,
            "kt": kt_c, "v": v_c, "v8": v8_c, "mask": mask,
        })
    return in_maps, (ak, gk, gv)


def numpy_core_kernel(m, scales):
    """Numpy mirror of the device dataflow for one core (layout validation)."""
    ak, gk, gv = scales
    f = np.float32
    f16 = np.float16
    escale = SCALE / (ak * ak)
    xT = m["xT"].astype(f).reshape(128, NT, NTOK).transpose(1, 0, 2).reshape(H, NTOK)

    def unw(w):
        return w.astype(f).reshape(128, NT, KPC).transpose(1, 0, 2).reshape(H, KPC)

    qT = (unw(m["wq"]).T @ xT).astype(f16).astype(f)      # [512 feat, 32 tok]
    kT = (unw(m["wk"]).T @ xT / gk).astype(f16).astype(f)
    v_tok = (xT.T @ unw(m["wv"]) / gv).astype(f16).astype(f)   # [32 tok, 512]

    pn_all = []
    for h in range(HPC):
        sn = kT[HD * h: HD * (h + 1), :].T @ qT[HD * h: HD * (h + 1), :]
        pn = np.exp(escale * (sn + m["mask"])).astype(f16).astype(f)
        pn_all.append(pn)

    out4 = np.zeros((128, HPC * NTOK), f)
    den = np.zeros((1, HPC * NTOK), f)
    for b in range(B):
        for h in range(HPC):
            col = NTOK * h + S * b
            KTbh = m["kt"][h * B + b].astype(f)                      # [hd, kv]
            scT = KTbh.T @ qT[HD * h: HD * (h + 1), S * b: S * b + S]  # [kv, 4]
            pr = np.exp(escale * scT).astype(f16).astype(f)
            Vb16 = m["v"][h * B + b].astype(f).reshape(128, NT16, HD)
            Vb8 = m["v8"][h * B + b].astype(f).reshape(128, NT - NT16, HD)
            o4 = np.zeros((HD, S), f)
            for t in range(NT):
                Vt = Vb16[:, t] if t < NT16 else Vb8[:, t - NT16]
                o4 += Vt.T @ pr[128 * t: 128 * (t + 1), :]
            o4 += v_tok[:, HD * h: HD * (h + 1)].T @ pn_all[h][:, S * b: S * (b + 1)]
            d = pr.sum(axis=0) + pn_all[h][:, S * b: S * (b + 1)].sum(axis=0)
            out4[:, col: col + S] = o4
            den[0, col: col + S] = d

    rec = 1.0 / den
    attnT = (out4 * rec).astype(f16).astype(f)            # [128, (h tok)]

    woc = m["wo"].astype(f).reshape(128, HPC, H).transpose(1, 0, 2).reshape(KPC, H)
    outT = np.zeros((H, NTOK), f)
    for oc in range(NT):
        acc = np.zeros((128, NTOK), f)
        for h in range(HPC):
            acc += woc[128 * h: 128 * (h + 1), 128 * oc: 128 * (oc + 1)].T \
                @ attnT[:, NTOK * h: NTOK * (h + 1)]
        outT[128 * oc: 128 * (oc + 1)] = acc
    # device o_all layout: [128, (oc, tok)]
    return np.ascontiguousarray(
        outT.reshape(NT, 128, NTOK).transpose(1, 0, 2)).reshape(128, NT * NTOK)


def host_unpack(o_all):
    """[128, (oc 32, tok 32)] -> [NTOK, H]"""
    a = o_all.reshape(128, NT, NTOK).transpose(1, 0, 2).reshape(H, NTOK)
    return a.T


_NC_CACHE = {}


def get_nc(scales):
    ak, gk, gv = scales
    key = (round(float(ak), 6), round(float(gk), 6), round(float(gv), 6))
    if key not in _NC_CACHE:
        _NC_CACHE[key] = build_nc(ak, gk, gv)
    return _NC_CACHE[key]


def run_on_hw(inputs, trace=False, trace_cores=None):
    position = int(inputs["position"])
    assert position == POS, position
    in_maps, scales = build_core_inputs(
        np.asarray(inputs["hidden_states"]), np.asarray(inputs["Wq"]),
        np.asarray(inputs["Wk"]), np.asarray(inputs["Wv"]), np.asarray(inputs["Wo"]),
        np.asarray(inputs["key_cache"]), np.asarray(inputs["value_cache"]))
    nc = get_nc(scales)
    res = run_bass_kernel_spmd(nc, in_maps, core_ids=list(range(N_CORES)),
                               trace=trace, trace_cores=trace_cores)
    partial = np.zeros((NTOK, H), np.float64)
    for c in range(N_CORES):
        partial += host_unpack(res.results[c]["out"].astype(np.float64))
    out = partial.astype(np.float32).reshape(B, S, H)
    return out, res


def kernel(**inputs) -> np.ndarray:
    out, _ = run_on_hw(inputs, trace=False)
    return out
